# revision 13
# baseline (speedup 1.0000x reference)
"""FRAUDRE InterAgg via two-pass DRAM->DRAM indirect DMA gathers.

Mechanism: a DRAM-dest indirect DMA (InstDMACopy on qPoolDynamic) with a
[1024, 64] f32 dest, coef=64 and a [128, 8] i32 offset AP consumes all 1024
index cells bijectively (dest row d <- cell q1(d), q1(d) = (d%2)*512 + (d%16)//2
+ 8*(d//16)) at ~1.1us/instruction -- ~8x cheaper per row than the SBUF-dest
path (128 rows / ~1.1us). Addressing granularity is 16 rows (addr = value*16
rows + element_offset/64 rows), so gathers are bucketed by row%16 with
element_offset selecting the low 4 bits.

Pipeline per core (1024 nodes, 97 rows/node = 96 neighbors + self):
  pass 1: 160 D2D gathers (10 blocks x 16 row%16-buckets, pads OOB-skipped)
          feats -> staging1 (bucket-ordered rows, host-assigned)
  pass 2: 104 D2D gathers staging1 -> staging2 in node-major
          (t, p, slot) order (slot padded 97->104)
  readback: HWDGE contiguous [128, 6656] per tile, DVE tree-reduce + relu +
          softmax(alpha) weighted sum (same math as the reference), store.
"""

import numpy as np


def _import_concourse():
    try:
        import concourse.bass  # noqa: F401
    except ImportError:
        import sys

        for p in ("/opt/trn_rl_repo", "/root/.axon_site/_ro/trn_rl_repo"):
            if p not in sys.path:
                sys.path.insert(0, p)
        import concourse.bass  # noqa: F401


N_CORES = 8
NUM_NODES = 1_000_000
EMBED = 64
N_BATCH = 8192
DEG = 32
PER_CORE = N_BATCH // N_CORES  # 1024
P = 128
N_TILES = PER_CORE // P  # 8

SLOTS = 1 + 3 * DEG  # 97 rows per node
SLOTS_PAD = 104  # pad to 13312 rows per tile
BLK = 2048  # rows per D2D instruction (D = 16*coef, coef=128 -> identity map)
COEF = 2 * EMBED  # 128
BUCK = 32  # addressing granularity = 32 rows -> bucket by row%32
NCLS = 26  # pass-2 residue classes (52 blocks, 2 per class)
P2_BLOCKS = N_TILES * P * SLOTS_PAD // BLK  # 52
STAG2_ROWS = P2_BLOCKS * BLK  # 106496
FEAT_ROWS = NUM_NODES + 16


def min_p1k(nodes_c, neighs_c):
    """Smallest pass-1 blocks-per-bucket K for this core's inputs."""
    frow = np.empty((PER_CORE, SLOTS), np.int64)
    frow[:, 0] = nodes_c
    for r in range(3):
        frow[:, 1 + DEG * r : 1 + DEG * (r + 1)] = neighs_c[r]
    frow = frow.ravel()
    g = np.repeat(np.arange(PER_CORE), SLOTS)
    slot = np.tile(np.arange(SLOTS), PER_CORE)
    t = g // P
    p = g % P
    s2 = t * (P * SLOTS_PAD) + p * SLOTS_PAD + slot
    u = (s2 // BLK) % NCLS
    b = frow % BUCK
    key = b * NCLS + u
    counts = np.bincount(key, minlength=BUCK * NCLS)
    return int(-(-counts.max() // 64))


def build_host_indices_v2(nodes_c, neighs_c, K):
    """Pass-1: bucket b (= feature row % 32) owns blocks jb = b*K + m.
    Pass-2 block i2 requires its sources at staging1 row % 32 == i2 % 26.
    Landing map at (D=2048, coef=128) is the identity: dest row d <- cell d.
    """
    frow = np.empty((PER_CORE, SLOTS), np.int64)
    frow[:, 0] = nodes_c
    for r in range(3):
        frow[:, 1 + DEG * r : 1 + DEG * (r + 1)] = neighs_c[r]
    frow = frow.ravel()

    g = np.repeat(np.arange(PER_CORE), SLOTS)
    slot = np.tile(np.arange(SLOTS), PER_CORE)
    t = g // P
    p = g % P
    s2 = t * (P * SLOTS_PAD) + p * SLOTS_PAD + slot
    i2 = s2 // BLK
    d2 = s2 % BLK
    u = i2 % NCLS  # required staging1 residue mod 32 (0..25)
    b = frow % BUCK

    key = b * NCLS + u
    order = np.argsort(key, kind="stable")
    sk = key[order]
    first = np.r_[True, sk[1:] != sk[:-1]]
    grp_start = np.where(first)[0]
    rank = np.arange(len(sk)) - np.repeat(grp_start, np.diff(np.r_[grp_start, len(sk)]))

    cap = 64 * K
    if rank.max() >= cap:
        raise RuntimeError(f"pass-1 overflow: max rank {rank.max()} >= {cap}")

    rr = rank
    m = rr // 64
    n = rr % 64
    jb = b[order] * K + m
    d1 = BUCK * n + u[order]
    s1 = jb * BLK + d1

    s1_mem = np.empty(len(s1), np.int64)
    s1_mem[order] = s1

    # pads are OOB-skipped via bounds_check (never generate drain traffic;
    # in-range pads would all hit one HBM row and serialize on its bank)
    cells1 = np.full((BUCK * K, BLK), ((NUM_NODES - 1) >> 5) + 1, np.int32)
    cells1[jb, d1] = (frow[order] >> 5).astype(np.int32)

    cells2 = np.full((P2_BLOCKS, BLK), ((BUCK * K * BLK - 1) >> 5) + 1, np.int32)
    cells2[i2, d2] = (s1_mem >> 5).astype(np.int32)

    i2_all = np.arange(P2_BLOCKS)
    eo1 = np.arange(BUCK * K) // K  # bucket of block
    eo2 = i2_all % NCLS

    return cells1, cells2, eo1.astype(np.int64), eo2.astype(np.int64)


def build_nc(K):
    _import_concourse()
    from contextlib import ExitStack

    import concourse.bass as bass
    import concourse.mybir as mybir

    f32 = mybir.dt.float32
    i32 = mybir.dt.int32
    Exp = mybir.ActivationFunctionType.Exp

    P1_NBLK = BUCK * K
    STAG1_ROWS = P1_NBLK * BLK
    nc = bass.Bass(num_swdge_queues=4, dynamic_dma_scratch_size=49152)
    feats = nc.dram_tensor("features", [FEAT_ROWS, EMBED], f32, kind="ExternalInput")
    alpha = nc.dram_tensor("alpha", [2 * EMBED, 3], f32, kind="ExternalInput")
    idx1 = nc.dram_tensor("idx1", [P, P1_NBLK * 16], i32, kind="ExternalInput")
    idx2 = nc.dram_tensor("idx2", [P, P2_BLOCKS * 16], i32, kind="ExternalInput")
    out = nc.dram_tensor("out", [PER_CORE, 3 * EMBED], f32, kind="ExternalOutput")
    stag1 = nc.dram_tensor("stag1", [STAG1_ROWS, EMBED], f32)
    stag2 = nc.dram_tensor("stag2", [STAG2_ROWS, EMBED], f32)
    w_dram = nc.dram_tensor("w_scratch", [2 * EMBED, 3], f32)

    def d2d(gp, out_ap, in_t, off_ap, eo_rows, bound_reg, q=0):
        o = gp.lower_ap_dma(out_ap, for_indirect_dma=True)
        i = gp.lower_ap_dma(in_t[:], for_indirect_dma=True)
        i.append(gp.lower_ap_dma(off_ap)[0])
        i[0].dynamic_ap_info = mybir.DynamicAccessPatternInfo(
            c=int(eo_rows) * EMBED,
            actual_ap=out_ap.ap,
            indirect_dim_max_index=in_t.shape[0],
            offset_expr=[
                mybir.DynamicAccessPatternOffsetExpr(
                    coef=COEF,
                    aff_expr=mybir.DynamicAccessPatternOffsetExprAffExpr(
                        kind="IndirectArgId", arg_id=1
                    ),
                )
            ],
        )
        return gp.add_instruction(
            mybir.InstDMACopy(
                name=gp.bass.get_next_instruction_name(),
                queue=f"qPoolDynamic{q or ''}",
                mode="Copy",
                ins=i + [bound_reg],
                outs=o,
                oob_is_err=False,
            )
        )

    from contextlib import ExitStack

    with ExitStack() as ctx:
        e = ctx.enter_context

        alpha_sb = e(nc.sbuf_tensor([2 * EMBED, 3], f32))
        w_e = e(nc.sbuf_tensor([2 * EMBED, 3], f32))
        w_s = e(nc.sbuf_tensor([2 * EMBED, 1], f32))
        w_rs = e(nc.sbuf_tensor([2 * EMBED, 1], f32))
        w_sb = e(nc.sbuf_tensor([2 * EMBED, 3], f32))
        wb_sb = e(nc.sbuf_tensor([P, 3 * EMBED], f32))
        idx1_sb = e(nc.sbuf_tensor([P, P1_NBLK * 16], i32))
        idx2_sb = e(nc.sbuf_tensor([P, P2_BLOCKS * 16], i32))
        rb = [e(nc.sbuf_tensor(f"rb{i}", [P, SLOTS_PAD * EMBED], f32)) for i in range(2)]
        out_sb = [e(nc.sbuf_tensor(f"out_sb{i}", [P, 3 * EMBED], f32)) for i in range(2)]
        rl = e(nc.sbuf_tensor([P, EMBED], f32))
        tmp = e(nc.sbuf_tensor([P, EMBED], f32))

        alpha_sem = e(nc.semaphore("alpha_sem"))
        idx_sem = e(nc.semaphore("idx_sem"))
        e_sem = e(nc.semaphore("e_sem"))
        v_sem = e(nc.semaphore("v_sem"))
        wd_sem = e(nc.semaphore("wd_sem"))
        wb_sem = e(nc.semaphore("wb_sem"))
        p1_sem = e(nc.semaphore("p1_sem"))
        p2_sems = [e(nc.semaphore(f"p2_{i}")) for i in range(4)]
        rb_sem = e(nc.semaphore("rb_sem"))
        dve_done = e(nc.semaphore("dve_done"))
        st_sem = e(nc.semaphore("st_sem"))

        block = e(nc.Block())

        @block.sync
        def _(sync):
            sync.dma_start(out=alpha_sb[:], in_=alpha[:, :]).then_inc(alpha_sem, 16)
            sync.dma_start(out=idx1_sb[:], in_=idx1[:, :]).then_inc(idx_sem, 16)
            sync.dma_start(out=idx2_sb[:], in_=idx2[:, :]).then_inc(idx_sem, 16)
            sync.wait_ge(v_sem, 1)
            sync.dma_start(out=w_dram[:, :], in_=w_sb[:]).then_inc(wd_sem, 16)
            for t in range(N_TILES):
                need = -(-(P * SLOTS_PAD * (t + 1)) // BLK)  # ceil(6.5*(t+1))
                for j in range(4):
                    # blocks [0, need) on queue j: i2 % 4 == j
                    nj = (need - j + 3) // 4
                    if nj > 0:
                        sync.wait_ge(p2_sems[j], 16 * nj)
                if t >= 2:
                    sync.wait_ge(dve_done, t - 1)
                sync.dma_start(
                    out=rb[t % 2][:],
                    in_=stag2[:, :]
                    .rearrange("(t p s) e -> t p (s e)", t=N_TILES, p=P)[t],
                ).then_inc(rb_sem, 16)
                if t >= 1:
                    sync.wait_ge(dve_done, t)
                    sync.dma_start(
                        out=out[(t - 1) * P : t * P, :], in_=out_sb[(t - 1) % 2][:]
                    ).then_inc(st_sem, 16)
            sync.wait_ge(dve_done, N_TILES)
            sync.dma_start(
                out=out[(N_TILES - 1) * P :, :], in_=out_sb[(N_TILES - 1) % 2][:]
            ).then_inc(st_sem, 16)

        @block.scalar
        def _(scalar):
            scalar.wait_ge(alpha_sem, 16)
            scalar.activation(w_e[:], alpha_sb[:], Exp).then_inc(e_sem, 1)

        @block.gpsimd
        def _(gpsimd):
            gpsimd.wait_ge(wd_sem, 16)
            gpsimd.dma_start(
                out=wb_sb[:],
                in_=w_dram[EMBED : 2 * EMBED, :]
                .rearrange("f r -> (f r)")[None, :]
                .partition_broadcast(P),
            ).then_inc(wb_sem, 16)
            b1 = gpsimd.lower_val_access(gpsimd.to_reg((NUM_NODES - 1) >> 5))
            b2 = gpsimd.lower_val_access(gpsimd.to_reg((STAG1_ROWS - 1) >> 5))
            gpsimd.wait_ge(idx_sem, 32)
            for jb in range(P1_NBLK):
                d2d(
                    gpsimd,
                    stag1[jb * BLK : (jb + 1) * BLK, :],
                    feats,
                    idx1_sb[:, jb * 16 : (jb + 1) * 16],
                    jb // K,
                    b1,
                    q=jb % 4,
                ).then_inc(p1_sem, 16)
            gpsimd.wait_ge(p1_sem, 16 * P1_NBLK)
            for i2 in range(P2_BLOCKS):
                d2d(
                    gpsimd,
                    stag2[i2 * BLK : (i2 + 1) * BLK, :],
                    stag1,
                    idx2_sb[:, i2 * 16 : (i2 + 1) * 16],
                    i2 % NCLS,
                    b2,
                    q=i2 % 4,
                ).then_inc(p2_sems[i2 % 4], 16)

        @block.vector
        def _(vector):
            vector.wait_ge(e_sem, 1)
            vector.reduce_sum(w_s[:], w_e[:], axis=mybir.AxisListType.X)
            vector.drain()
            vector.reciprocal(w_rs[:], w_s[:])
            vector.drain()
            vector.tensor_mul(w_sb[:], w_e[:], w_rs[:].to_broadcast([2 * EMBED, 3]))
            vector.drain()
            vector.tensor_scalar_mul(w_sb[:], w_sb[:], 1.0 / DEG).then_inc(v_sem, 1)
            vector.wait_ge(wb_sem, 16)
            for t in range(N_TILES):
                buf = rb[t % 2]
                ob = out_sb[t % 2]
                vector.wait_ge(rb_sem, 16 * (t + 1))
                if t >= 2:
                    vector.wait_ge(st_sem, 16 * (t - 1))
                vector.tensor_copy(ob[:, 0:EMBED], buf[:, 0:EMBED])
                vector.drain()
                vector.tensor_relu(ob[:, EMBED : 2 * EMBED], ob[:, 0:EMBED])
                acc = ob[:, 2 * EMBED : 3 * EMBED]
                for r in range(3):
                    base = (1 + DEG * r) * EMBED
                    width = DEG * EMBED
                    first = True
                    while width > EMBED:
                        half = width // 2
                        if not first:
                            vector.drain()
                        vector.tensor_add(
                            buf[:, base : base + half],
                            buf[:, base : base + half],
                            buf[:, base + half : base + width],
                        )
                        width = half
                        first = False
                    vector.drain()
                    vector.tensor_relu(rl[:], buf[:, base : base + EMBED])
                    vector.drain()
                    wb_r = wb_sb[:, r : 3 * EMBED : 3]
                    if r == 0:
                        vector.tensor_mul(acc, rl[:], wb_r)
                    elif r == 1:
                        vector.tensor_mul(tmp[:], rl[:], wb_r)
                        vector.drain()
                        vector.tensor_add(acc, acc, tmp[:])
                    else:
                        vector.tensor_mul(tmp[:], rl[:], wb_r)
                        vector.drain()
                        vector.tensor_add(acc, acc, tmp[:]).then_inc(dve_done, 1)

    return nc


_NC_CACHE = {}


def _get_nc(K):
    if K not in _NC_CACHE:
        _NC_CACHE[K] = build_nc(K)
    return _NC_CACHE[K]


def _run(inputs, trace=False, trace_kwargs=None):
    _import_concourse()
    from concourse.bass_utils import run_bass_kernel_spmd

    features = np.asarray(inputs["features"], dtype=np.float32)
    feats_pad = np.concatenate(
        [features, np.zeros((16, EMBED), np.float32)], axis=0
    )
    feats_pad = np.ascontiguousarray(feats_pad)
    alpha = np.ascontiguousarray(np.asarray(inputs["alpha"], dtype=np.float32))
    nodes = np.asarray(inputs["nodes"]).astype(np.int64)
    nis = [np.asarray(inputs[f"neigh_idx{r + 1}"]).astype(np.int64) for r in range(3)]

    K = max(
        min_p1k(nodes[slice(c * PER_CORE, (c + 1) * PER_CORE)],
                [ni[c * PER_CORE : (c + 1) * PER_CORE] for ni in nis])
        for c in range(N_CORES)
    )
    nc = _get_nc(K)
    in_maps = []
    for c in range(N_CORES):
        sl = slice(c * PER_CORE, (c + 1) * PER_CORE)
        cells1, cells2, eo1, eo2 = build_host_indices_v2(
            nodes[sl], [ni[sl] for ni in nis], K
        )
        # cells arrays are [nblk, 1024]; SBUF layout [128, nblk*8]:
        # block jb cell q -> partition q//8, col jb*8 + q%8
        i1 = cells1.reshape(-1, P, 16).transpose(1, 0, 2).reshape(P, -1)
        i2 = cells2.reshape(P2_BLOCKS, P, 16).transpose(1, 0, 2).reshape(P, P2_BLOCKS * 16)
        m = {
            "features": feats_pad,
            "alpha": alpha,
            "idx1": np.ascontiguousarray(i1),
            "idx2": np.ascontiguousarray(i2),
        }
        in_maps.append(m)

    kw = {}
    if trace:
        kw["trace"] = True
        if trace_kwargs:
            kw.update(trace_kwargs)
    res = run_bass_kernel_spmd(nc, in_maps, list(range(N_CORES)), **kw)
    out_full = np.concatenate([res.results[c]["out"] for c in range(N_CORES)], axis=0)
    return out_full, res


def kernel(**inputs) -> np.ndarray:
    out, _ = _run(inputs)
    return out


# revision 15
# speedup vs baseline: 1.2038x; 1.2038x over previous
"""FRAUDRE InterAgg via two-pass DRAM->DRAM indirect DMA gathers.

Mechanism: a DRAM-dest indirect DMA (InstDMACopy on qPoolDynamic) with a
[1024, 64] f32 dest, coef=64 and a [128, 8] i32 offset AP consumes all 1024
index cells bijectively (dest row d <- cell q1(d), q1(d) = (d%2)*512 + (d%16)//2
+ 8*(d//16)) at ~1.1us/instruction -- ~8x cheaper per row than the SBUF-dest
path (128 rows / ~1.1us). Addressing granularity is 16 rows (addr = value*16
rows + element_offset/64 rows), so gathers are bucketed by row%16 with
element_offset selecting the low 4 bits.

Pipeline per core (1024 nodes, 97 rows/node = 96 neighbors + self):
  pass 1: 160 D2D gathers (10 blocks x 16 row%16-buckets, pads OOB-skipped)
          feats -> staging1 (bucket-ordered rows, host-assigned)
  pass 2: 104 D2D gathers staging1 -> staging2 in node-major
          (t, p, slot) order (slot padded 97->104)
  readback: HWDGE contiguous [128, 6656] per tile, DVE tree-reduce + relu +
          softmax(alpha) weighted sum (same math as the reference), store.
"""

import numpy as np


def _import_concourse():
    try:
        import concourse.bass  # noqa: F401
    except ImportError:
        import sys

        for p in ("/opt/trn_rl_repo", "/root/.axon_site/_ro/trn_rl_repo"):
            if p not in sys.path:
                sys.path.insert(0, p)
        import concourse.bass  # noqa: F401


N_CORES = 8
NUM_NODES = 1_000_000
EMBED = 64
N_BATCH = 8192
DEG = 32
PER_CORE = N_BATCH // N_CORES  # 1024
P = 128
N_TILES = PER_CORE // P  # 8

SLOTS = 1 + 3 * DEG  # 97 rows per node
SLOTS_PAD = 104  # pad to 13312 rows per tile
BLK = 2048  # rows per D2D instruction (D = 16*coef, coef=128 -> identity map)
COEF = 2 * EMBED  # 128
BUCK = 32  # addressing granularity = 32 rows -> bucket by row%32
NCLS = 26  # pass-2 residue classes (52 blocks, 2 per class)
P2_BLOCKS = N_TILES * P * SLOTS_PAD // BLK  # 52
STAG2_ROWS = P2_BLOCKS * BLK  # 106496
FEAT_ROWS = NUM_NODES + 16


def min_p1k(nodes_c, neighs_c):
    """Smallest pass-1 blocks-per-bucket K for this core's inputs."""
    frow = np.empty((PER_CORE, SLOTS), np.int64)
    frow[:, 0] = nodes_c
    for r in range(3):
        frow[:, 1 + DEG * r : 1 + DEG * (r + 1)] = neighs_c[r]
    frow = frow.ravel()
    g = np.repeat(np.arange(PER_CORE), SLOTS)
    slot = np.tile(np.arange(SLOTS), PER_CORE)
    t = g // P
    p = g % P
    s2 = t * (P * SLOTS_PAD) + p * SLOTS_PAD + slot
    u = (s2 // BLK) % NCLS
    b = frow % BUCK
    key = b * NCLS + u
    counts = np.bincount(key, minlength=BUCK * NCLS)
    return int(-(-counts.max() // 64))


def build_host_indices_v2(nodes_c, neighs_c, K):
    """Pass-1: bucket b (= feature row % 32) owns blocks jb = b*K + m.
    Pass-2 block i2 requires its sources at staging1 row % 32 == i2 % 26.
    Landing map at (D=2048, coef=128) is the identity: dest row d <- cell d.
    """
    frow = np.empty((PER_CORE, SLOTS), np.int64)
    frow[:, 0] = nodes_c
    for r in range(3):
        frow[:, 1 + DEG * r : 1 + DEG * (r + 1)] = neighs_c[r]
    frow = frow.ravel()

    g = np.repeat(np.arange(PER_CORE), SLOTS)
    slot = np.tile(np.arange(SLOTS), PER_CORE)
    t = g // P
    p = g % P
    s2 = t * (P * SLOTS_PAD) + p * SLOTS_PAD + slot
    i2 = s2 // BLK
    d2 = s2 % BLK
    u = i2 % NCLS  # required staging1 residue mod 32 (0..25)
    b = frow % BUCK

    key = b * NCLS + u
    order = np.argsort(key, kind="stable")
    sk = key[order]
    first = np.r_[True, sk[1:] != sk[:-1]]
    grp_start = np.where(first)[0]
    rank = np.arange(len(sk)) - np.repeat(grp_start, np.diff(np.r_[grp_start, len(sk)]))

    cap = 64 * K
    if rank.max() >= cap:
        raise RuntimeError(f"pass-1 overflow: max rank {rank.max()} >= {cap}")

    rr = rank
    m = rr // 64
    n = rr % 64
    jb = b[order] * K + m
    d1 = BUCK * n + u[order]
    s1 = jb * BLK + d1

    s1_mem = np.empty(len(s1), np.int64)
    s1_mem[order] = s1

    # pads are OOB-skipped via bounds_check (never generate drain traffic;
    # in-range pads would all hit one HBM row and serialize on its bank)
    cells1 = np.full((BUCK * K, BLK), ((NUM_NODES - 1) >> 5) + 1, np.int32)
    cells1[jb, d1] = (frow[order] >> 5).astype(np.int32)

    cells2 = np.full((P2_BLOCKS, BLK), ((BUCK * K * BLK - 1) >> 5) + 1, np.int32)
    cells2[i2, d2] = (s1_mem >> 5).astype(np.int32)

    i2_all = np.arange(P2_BLOCKS)
    eo1 = np.arange(BUCK * K) // K  # bucket of block
    eo2 = i2_all % NCLS

    return cells1, cells2, eo1.astype(np.int64), eo2.astype(np.int64)


def build_nc(K):
    _import_concourse()
    from contextlib import ExitStack

    import concourse.bass as bass
    import concourse.mybir as mybir

    f32 = mybir.dt.float32
    i32 = mybir.dt.int32
    Exp = mybir.ActivationFunctionType.Exp

    P1_NBLK = BUCK * K
    STAG1_ROWS = P1_NBLK * BLK
    nc = bass.Bass(num_swdge_queues=4, dynamic_dma_scratch_size=49152)
    feats = nc.dram_tensor("features", [FEAT_ROWS, EMBED], f32, kind="ExternalInput")
    alpha = nc.dram_tensor("alpha", [2 * EMBED, 3], f32, kind="ExternalInput")
    idx1 = nc.dram_tensor("idx1", [P, P1_NBLK * 16], i32, kind="ExternalInput")
    idx2 = nc.dram_tensor("idx2", [P, P2_BLOCKS * 16], i32, kind="ExternalInput")
    out = nc.dram_tensor("out", [PER_CORE, 3 * EMBED], f32, kind="ExternalOutput")
    stag1 = nc.dram_tensor("stag1", [STAG1_ROWS, EMBED], f32)
    stag2 = nc.dram_tensor("stag2", [STAG2_ROWS, EMBED], f32)
    w_dram = nc.dram_tensor("w_scratch", [2 * EMBED, 3], f32)

    def d2d(gp, out_ap, in_t, off_ap, eo_rows, bound_reg, q=0):
        o = gp.lower_ap_dma(out_ap, for_indirect_dma=True)
        i = gp.lower_ap_dma(in_t[:], for_indirect_dma=True)
        i.append(gp.lower_ap_dma(off_ap)[0])
        i[0].dynamic_ap_info = mybir.DynamicAccessPatternInfo(
            c=int(eo_rows) * EMBED,
            actual_ap=out_ap.ap,
            indirect_dim_max_index=in_t.shape[0],
            offset_expr=[
                mybir.DynamicAccessPatternOffsetExpr(
                    coef=COEF,
                    aff_expr=mybir.DynamicAccessPatternOffsetExprAffExpr(
                        kind="IndirectArgId", arg_id=1
                    ),
                )
            ],
        )
        return gp.add_instruction(
            mybir.InstDMACopy(
                name=gp.bass.get_next_instruction_name(),
                queue=f"qPoolDynamic{q or ''}",
                mode="Copy",
                ins=i + [bound_reg],
                outs=o,
                oob_is_err=False,
            )
        )

    from contextlib import ExitStack

    with ExitStack() as ctx:
        e = ctx.enter_context

        alpha_sb = e(nc.sbuf_tensor([2 * EMBED, 3], f32))
        w_e = e(nc.sbuf_tensor([2 * EMBED, 3], f32))
        w_s = e(nc.sbuf_tensor([2 * EMBED, 1], f32))
        w_rs = e(nc.sbuf_tensor([2 * EMBED, 1], f32))
        w_sb = e(nc.sbuf_tensor([2 * EMBED, 3], f32))
        wb_sb = e(nc.sbuf_tensor([P, 3 * EMBED], f32))
        idx1_sb = e(nc.sbuf_tensor([P, P1_NBLK * 16], i32))
        idx2_sb = e(nc.sbuf_tensor([P, P2_BLOCKS * 16], i32))
        rb = [e(nc.sbuf_tensor(f"rb{i}", [P, SLOTS_PAD * EMBED], f32)) for i in range(2)]
        out_sb = [e(nc.sbuf_tensor(f"out_sb{i}", [P, 3 * EMBED], f32)) for i in range(2)]
        rl = e(nc.sbuf_tensor([P, EMBED], f32))
        tmp = e(nc.sbuf_tensor([P, EMBED], f32))

        alpha_sem = e(nc.semaphore("alpha_sem"))
        idx_sem = e(nc.semaphore("idx_sem"))
        e_sem = e(nc.semaphore("e_sem"))
        v_sem = e(nc.semaphore("v_sem"))
        wd_sem = e(nc.semaphore("wd_sem"))
        wb_sem = e(nc.semaphore("wb_sem"))
        p1_sem = e(nc.semaphore("p1_sem"))
        p2_sems = [e(nc.semaphore(f"p2_{i}")) for i in range(2)]
        rb_sem = e(nc.semaphore("rb_sem"))
        dve_done = e(nc.semaphore("dve_done"))
        st_sem = e(nc.semaphore("st_sem"))

        block = e(nc.Block())

        @block.sync
        def _(sync):
            sync.dma_start(out=idx1_sb[:], in_=idx1[:, :]).then_inc(idx_sem, 16)
            sync.dma_start(out=alpha_sb[:], in_=alpha[:, :]).then_inc(alpha_sem, 16)
            sync.dma_start(out=idx2_sb[:], in_=idx2[:, :]).then_inc(idx_sem, 16)
            sync.wait_ge(v_sem, 1)
            sync.dma_start(out=w_dram[:, :], in_=w_sb[:]).then_inc(wd_sem, 16)
            for t in range(N_TILES):
                need = -(-(P * SLOTS_PAD * (t + 1)) // BLK)  # ceil(6.5*(t+1))
                sync.wait_ge(p2_sems[0], 16 * ((need + 1) // 2))
                sync.wait_ge(p2_sems[1], 16 * (need // 2))
                if t >= 2:
                    sync.wait_ge(dve_done, t - 1)
                sync.dma_start(
                    out=rb[t % 2][:],
                    in_=stag2[:, :]
                    .rearrange("(t p s) e -> t p (s e)", t=N_TILES, p=P)[t],
                ).then_inc(rb_sem, 16)
                if t >= 1:
                    sync.wait_ge(dve_done, t)
                    sync.dma_start(
                        out=out[(t - 1) * P : t * P, :], in_=out_sb[(t - 1) % 2][:]
                    ).then_inc(st_sem, 16)
            sync.wait_ge(dve_done, N_TILES)
            sync.dma_start(
                out=out[(N_TILES - 1) * P :, :], in_=out_sb[(N_TILES - 1) % 2][:]
            ).then_inc(st_sem, 16)

        @block.scalar
        def _(scalar):
            scalar.wait_ge(alpha_sem, 16)
            scalar.activation(w_e[:], alpha_sb[:], Exp).then_inc(e_sem, 1)

        @block.gpsimd
        def _(gpsimd):
            b1 = gpsimd.lower_val_access(gpsimd.to_reg((NUM_NODES - 1) >> 5))
            b2 = gpsimd.lower_val_access(gpsimd.to_reg((STAG1_ROWS - 1) >> 5))
            # sync issues idx1 before idx2 on the same HWDGE queue (in-order),
            # so idx_sem >= 16 implies the idx1 load is complete
            gpsimd.wait_ge(idx_sem, 16)
            for jb in range(P1_NBLK):
                d2d(
                    gpsimd,
                    stag1[jb * BLK : (jb + 1) * BLK, :],
                    feats,
                    idx1_sb[:, jb * 16 : (jb + 1) * 16],
                    jb // K,
                    b1,
                    q=jb % 4,
                ).then_inc(p1_sem, 16)
            gpsimd.wait_ge(wd_sem, 16)
            gpsimd.dma_start(
                out=wb_sb[:],
                in_=w_dram[EMBED : 2 * EMBED, :]
                .rearrange("f r -> (f r)")[None, :]
                .partition_broadcast(P),
            ).then_inc(wb_sem, 16)
            gpsimd.wait_ge(idx_sem, 32)
            gpsimd.wait_ge(p1_sem, 16 * P1_NBLK)
            for i2 in range(P2_BLOCKS):
                d2d(
                    gpsimd,
                    stag2[i2 * BLK : (i2 + 1) * BLK, :],
                    stag1,
                    idx2_sb[:, i2 * 16 : (i2 + 1) * 16],
                    i2 % NCLS,
                    b2,
                    q=i2 % 2,
                ).then_inc(p2_sems[i2 % 2], 16)

        @block.vector
        def _(vector):
            vector.wait_ge(e_sem, 1)
            vector.reduce_sum(w_s[:], w_e[:], axis=mybir.AxisListType.X)
            vector.drain()
            vector.reciprocal(w_rs[:], w_s[:])
            vector.drain()
            vector.tensor_mul(w_sb[:], w_e[:], w_rs[:].to_broadcast([2 * EMBED, 3]))
            vector.drain()
            vector.tensor_scalar_mul(w_sb[:], w_sb[:], 1.0 / DEG).then_inc(v_sem, 1)
            vector.wait_ge(wb_sem, 16)
            for t in range(N_TILES):
                buf = rb[t % 2]
                ob = out_sb[t % 2]
                vector.wait_ge(rb_sem, 16 * (t + 1))
                if t >= 2:
                    vector.wait_ge(st_sem, 16 * (t - 1))
                vector.tensor_copy(ob[:, 0:EMBED], buf[:, 0:EMBED])
                vector.drain()
                vector.tensor_relu(ob[:, EMBED : 2 * EMBED], ob[:, 0:EMBED])
                acc = ob[:, 2 * EMBED : 3 * EMBED]
                for r in range(3):
                    base = (1 + DEG * r) * EMBED
                    width = DEG * EMBED
                    first = True
                    while width > EMBED:
                        half = width // 2
                        if not first:
                            vector.drain()
                        vector.tensor_add(
                            buf[:, base : base + half],
                            buf[:, base : base + half],
                            buf[:, base + half : base + width],
                        )
                        width = half
                        first = False
                    vector.drain()
                    vector.tensor_relu(rl[:], buf[:, base : base + EMBED])
                    vector.drain()
                    wb_r = wb_sb[:, r : 3 * EMBED : 3]
                    if r == 0:
                        vector.tensor_mul(acc, rl[:], wb_r)
                    elif r == 1:
                        vector.tensor_mul(tmp[:], rl[:], wb_r)
                        vector.drain()
                        vector.tensor_add(acc, acc, tmp[:])
                    else:
                        vector.tensor_mul(tmp[:], rl[:], wb_r)
                        vector.drain()
                        vector.tensor_add(acc, acc, tmp[:]).then_inc(dve_done, 1)

    return nc


_NC_CACHE = {}


def _get_nc(K):
    if K not in _NC_CACHE:
        _NC_CACHE[K] = build_nc(K)
    return _NC_CACHE[K]


def _run(inputs, trace=False, trace_kwargs=None):
    _import_concourse()
    from concourse.bass_utils import run_bass_kernel_spmd

    features = np.asarray(inputs["features"], dtype=np.float32)
    feats_pad = np.concatenate(
        [features, np.zeros((16, EMBED), np.float32)], axis=0
    )
    feats_pad = np.ascontiguousarray(feats_pad)
    alpha = np.ascontiguousarray(np.asarray(inputs["alpha"], dtype=np.float32))
    nodes = np.asarray(inputs["nodes"]).astype(np.int64)
    nis = [np.asarray(inputs[f"neigh_idx{r + 1}"]).astype(np.int64) for r in range(3)]

    K = max(
        min_p1k(nodes[slice(c * PER_CORE, (c + 1) * PER_CORE)],
                [ni[c * PER_CORE : (c + 1) * PER_CORE] for ni in nis])
        for c in range(N_CORES)
    )
    nc = _get_nc(K)
    in_maps = []
    for c in range(N_CORES):
        sl = slice(c * PER_CORE, (c + 1) * PER_CORE)
        cells1, cells2, eo1, eo2 = build_host_indices_v2(
            nodes[sl], [ni[sl] for ni in nis], K
        )
        # cells arrays are [nblk, 1024]; SBUF layout [128, nblk*8]:
        # block jb cell q -> partition q//8, col jb*8 + q%8
        i1 = cells1.reshape(-1, P, 16).transpose(1, 0, 2).reshape(P, -1)
        i2 = cells2.reshape(P2_BLOCKS, P, 16).transpose(1, 0, 2).reshape(P, P2_BLOCKS * 16)
        m = {
            "features": feats_pad,
            "alpha": alpha,
            "idx1": np.ascontiguousarray(i1),
            "idx2": np.ascontiguousarray(i2),
        }
        in_maps.append(m)

    kw = {}
    if trace:
        kw["trace"] = True
        if trace_kwargs:
            kw.update(trace_kwargs)
    res = run_bass_kernel_spmd(nc, in_maps, list(range(N_CORES)), **kw)
    out_full = np.concatenate([res.results[c]["out"] for c in range(N_CORES)], axis=0)
    return out_full, res


def kernel(**inputs) -> np.ndarray:
    out, _ = _run(inputs)
    return out
